# revision 14
# baseline (speedup 1.0000x reference)
"""Spatial attention block (GroupNorm + QKV 1x1 + full spatial attention +
out-proj + residual) on 8 Trainium2 NeuronCores.

Sharding: core = (batch b, spatial quarter j). Each core receives its batch
image rotated along the flattened spatial axis by -1024*j, so the SPMD
program always computes attention outputs for "the first 1024 query
positions" of its input. Attention is invariant to a joint rotation of the
K/V spatial axis, and GroupNorm stats are rotation-invariant, so the host
just concatenates the per-core [256, 1024] outputs.

v2b: fp8(e4m3) everywhere it pays. QKV projections contract 256 channels
per pass via DoubleRow (fp8 xn, fp8 weights); AV contracts 256 keys per
pass via DoubleRow (fp8 e, fp8 [V^T|1]); softmax exp split across ScalarE
(LUT exp -> fp8) and VectorE (Schraudolph int8 bit-trick -> fp8) at
512-column granularity so score PSUM banks recycle fast; reciprocal via
Ln/Exp on ScalarE, batched per head pair to minimize ACT table switches.
"""

import sys

for _p in ("/opt/trn_rl_repo", "/root/.axon_site/_ro/trn_rl_repo"):
    if _p not in sys.path:
        sys.path.insert(0, _p)

import numpy as np

import concourse.bacc as bacc
import concourse.bass as bass
import concourse.tile as tile
from concourse import mybir
from concourse.bass_utils import run_bass_kernel_spmd

F32 = mybir.dt.float32
F32R = mybir.dt.float32r
BF16 = mybir.dt.bfloat16
FP8 = mybir.dt.float8e4
I8 = mybir.dt.int8
U8 = mybir.dt.uint8
AF = mybir.ActivationFunctionType
DR = mybir.MatmulPerfMode.DoubleRow

B, C, H, W = 2, 256, 64, 64
S = H * W              # 4096 spatial positions
NH = 4                 # heads
HD = C // NH           # 64 head dim
NQ = S // 4            # 1024 query positions per core
NCHUNK = S // 128      # 32 key chunks
NCP = NCHUNK // 2      # 16 chunk pairs (DoubleRow contracts 256 keys/pass)
EPS = 1e-5
SCALE = 1.0 / 16.0     # 1/sqrt(C)
WV = 68                # per-head stride in the [V^T | ones] tile (64 V + 1 one + 3 pad)
# Schraudolph int8 fp8e4m3 exp: i8 = round(score * SCH_A + SCH_B)
SCH_A = 8.0 * 1.4426950408889634 / 16.0   # 8*log2(e)*SCALE
SCH_B = 56.0                               # 7 (exp bias) * 8


def _build_program():
    nc = bacc.Bacc(None)

    x_d = nc.declare_dram_parameter("x", [C, S], F32, isOutput=False)
    wqkv8_d = nc.declare_dram_parameter("wqkv8", [128, 2 * 3 * C], FP8, isOutput=False)
    woutT_d = nc.declare_dram_parameter("woutT", [NH, HD, C], BF16, isOutput=False)
    gnw_d = nc.declare_dram_parameter("gnw", [2, 128, 1], F32, isOutput=False)
    gnb_d = nc.declare_dram_parameter("gnb", [2, 128, 1], F32, isOutput=False)
    ob_d = nc.declare_dram_parameter("ob", [2, 128, 1], F32, isOutput=False)
    gsel_d = nc.declare_dram_parameter("gsel", [128, 8], F32R, isOutput=False)
    gselT_d = nc.declare_dram_parameter("gselT", [8, 128], F32R, isOutput=False)
    y_d = nc.declare_dram_parameter("y", [C, NQ], F32, isOutput=True)

    with tile.TileContext(nc) as tc, nc.allow_low_precision("fp8 matmul inputs"):
        _emit(nc, tc, x_d, wqkv8_d, woutT_d, gnw_d, gnb_d, ob_d, gsel_d, gselT_d, y_d)
    nc.finalize()
    return nc


def _emit(nc, tc, x_d, wqkv8_d, woutT_d, gnw_d, gnb_d, ob_d, gsel_d, gselT_d, y_d):
    from contextlib import ExitStack

    ctx = ExitStack()
    with ctx:
        persist = ctx.enter_context(tc.tile_pool(name="persist", bufs=1))
        pp = ctx.enter_context(tc.tile_pool(name="pp", bufs=4, space="PSUM"))
        po = ctx.enter_context(tc.tile_pool(name="po", bufs=2, space="PSUM"))
        ep = ctx.enter_context(tc.tile_pool(name="epool", bufs=2))

        # ---- persistent SBUF tiles -------------------------------------
        x_sb = [persist.tile([128, S], F32, tag=f"x{t}", name=f"x{t}") for t in range(2)]
        k_sb = [persist.tile([128, S], BF16, tag=f"k{t}", name=f"k{t}") for t in range(2)]
        q_sb = [persist.tile([128, NQ], BF16, tag=f"q{t}", name=f"q{t}") for t in range(2)]
        # [V^T | 1] per chunk-pair: cols (j, h*WV + c) j=key subchunk, h=head
        vt_sb = [
            persist.tile([128, 2 * NH * WV], FP8, tag=f"vt{c}", name=f"vt{c}")
            for c in range(NCP)
        ]
        attn_sb = [persist.tile([64, NQ], BF16, tag=f"at{h}", name=f"at{h}") for h in range(NH)]
        wq8_sb = persist.tile([128, 2 * 3 * C], FP8, tag="wq8")
        wo_sb = [persist.tile([HD, C], BF16, tag=f"wo{ct}", name=f"wo{ct}") for ct in range(NH)]
        gnw_sb = [persist.tile([128, 1], F32, tag=f"gw{t}", name=f"gw{t}") for t in range(2)]
        gnb_sb = [persist.tile([128, 1], F32, tag=f"gb{t}", name=f"gb{t}") for t in range(2)]
        ob_sb = [persist.tile([128, 1], F32, tag=f"obias{t}", name=f"obias{t}") for t in range(2)]
        gsel_sb = persist.tile([128, 8], F32R, tag="gsel")
        gselT_sb = persist.tile([8, 128], F32R, tag="gselT")
        oacc_sb = [
            persist.tile([128, NQ], F32, tag=f"oacc{t}", name=f"oacc{t}")
            for t in range(2)
        ]
        oh_sb = [persist.tile([65, NQ], F32, tag=f"oh{h}", name=f"oh{h}") for h in range(NH)]
        rs_sb = [persist.tile([1, NQ], F32R, tag=f"rs{h}", name=f"rs{h}") for h in range(NH)]
        ln_sb = [persist.tile([1, NQ], F32, tag=f"ln{h}", name=f"ln{h}") for h in range(2, NH)]
        # pair-0 reciprocal rows broadcast to 64 partitions by GpSimd
        rsb_sb = [persist.tile([64, NQ], F32, tag=f"rsb{h}", name=f"rsb{h}") for h in range(2)]
        eps_sb = persist.tile([128, 1], F32, tag="eps")
        nc.vector.memset(eps_sb, EPS)
        # memset only supports standard value types: build ones as F32 and
        # bitcast to F32R at the use site.
        ones64_f32 = persist.tile([1, 64], F32, tag="ones64")
        nc.vector.memset(ones64_f32, 1.0)
        ones64_sb = ones64_f32.bitcast(F32R)

        for t in range(2):
            for xc in range(4):
                eng = nc.sync if xc % 2 == 0 else nc.gpsimd
                eng.dma_start(
                    out=x_sb[t][:, 1024 * xc : 1024 * (xc + 1)],
                    in_=x_d[128 * t : 128 * (t + 1), 1024 * xc : 1024 * (xc + 1)],
                )
            nc.sync.dma_start(out=gnw_sb[t], in_=gnw_d[t])
            nc.sync.dma_start(out=gnb_sb[t], in_=gnb_d[t])
            nc.sync.dma_start(out=ob_sb[t], in_=ob_d[t])
        nc.sync.dma_start(out=wq8_sb, in_=wqkv8_d[:, :])
        for ct in range(NH):
            nc.sync.dma_start(out=wo_sb[ct], in_=woutT_d[ct])
        nc.sync.dma_start(out=gsel_sb, in_=gsel_d[:])
        nc.sync.dma_start(out=gselT_sb, in_=gselT_d[:])
        for oct_ in range(2):
            nc.vector.tensor_scalar(
                out=oacc_sb[oct_],
                in0=x_sb[oct_][:, 0:NQ],
                scalar1=ob_sb[oct_],
                scalar2=None,
                op0=mybir.AluOpType.add,
            )

        # ones columns of the [V^T | ones] tiles (0x38 is fp8e4m3 1.0)
        for c in range(NCP):
            ones_cols = vt_sb[c].rearrange("p (j h x) -> p j h x", j=2, h=NH)[
                :, :, :, HD : HD + 1
            ]
            nc.vector.memset(ones_cols.bitcast(U8), 0x38)

        wq8r = wq8_sb.rearrange("p (j o) -> p j o", j=2)

        warm_n = [0]

        def emit_warm(n):
            d = po.tile([128, 512], F32, tag="po", name=f"warm{warm_n[0]}")
            warm_n[0] += 1
            for _ in range(n):
                nc.tensor.matmul(
                    d,
                    wq8r[:, :, 0:128],
                    wq8r[:, :, 0:512],
                    start=True,
                    stop=True,
                    perf_mode=DR,
                )

        emit_warm(12)

        # ---- GroupNorm -------------------------------------------------
        # per-channel stats via bn_stats (free-dim), then combine the 16
        # channels of each group across partitions with small PE matmuls.
        # Ln/Exp batched across both channel-halves (one table-set trip).
        with tc.tile_pool(name="gnpool", bufs=1) as gnp, tc.tile_pool(
            name="xn", bufs=1
        ) as xnp:
            xn8_sb = xnp.tile([128, 2 * S], FP8, tag="xn8")
            xn8r = xn8_sb.rearrange("p (j s) -> p j s", j=2)
            vg_t, mr_t = [], []
            for t in range(2):
                nsub = S // 512
                st6 = gnp.tile([128, nsub, 6], F32, tag=f"st6_{t}")
                for i in range(nsub):
                    nc.vector.bn_stats(
                        out=st6[:, i, :], in_=x_sb[t][:, 512 * i : 512 * (i + 1)]
                    )
                mv = gnp.tile([128, 2], F32, tag=f"mv{t}")
                nc.vector.bn_aggr(out=mv, in_=st6)
                # stats2 = [mean, var + mean^2]  (per channel)
                stats2 = gnp.tile([128, 2], F32R, tag=f"s2_{t}")
                nc.vector.tensor_copy(out=stats2[:, 0:1], in_=mv[:, 0:1])
                nc.vector.tensor_tensor(
                    out=stats2[:, 1:2],
                    in0=mv[:, 0:1],
                    in1=mv[:, 0:1],
                    op=mybir.AluOpType.mult,
                )
                nc.vector.tensor_tensor(
                    out=stats2[:, 1:2],
                    in0=stats2[:, 1:2],
                    in1=mv[:, 1:2],
                    op=mybir.AluOpType.add,
                )
                # group sums: [8, 2] = gsel.T @ stats2, then /16
                pg = pp.tile([8, 2], F32, tag="pp")
                nc.tensor.matmul(pg, (gsel_sb), (stats2), start=True, stop=True)
                g2 = gnp.tile([8, 2], F32, tag=f"g2_{t}")
                nc.scalar.activation(out=g2, in_=pg, func=AF.Copy, scale=1.0 / 16.0)
                # var_g = m2_g - mu_g^2
                mr = gnp.tile([8, 2], F32R, tag=f"mr{t}")
                nc.vector.tensor_copy(out=mr[:, 0:1], in_=g2[:, 0:1])
                vg = gnp.tile([8, 1], F32, tag=f"vg{t}")
                nc.vector.tensor_tensor(
                    out=vg, in0=g2[:, 0:1], in1=g2[:, 0:1], op=mybir.AluOpType.mult
                )
                nc.vector.tensor_tensor(
                    out=vg, in0=g2[:, 1:2], in1=vg, op=mybir.AluOpType.subtract
                )
                vg_t.append(vg)
                mr_t.append(mr)
            emit_warm(8)
            # rstd = exp(-0.5*ln(var+eps)); batch Ln then Exp
            for t in range(2):
                nc.scalar.activation(out=vg_t[t], in_=vg_t[t], func=AF.Ln, bias=eps_sb[0:8])
            for t in range(2):
                nc.scalar.activation(out=mr_t[t][:, 1:2], in_=vg_t[t], func=AF.Exp, scale=-0.5)
            s_t, b_t = [], []
            for t in range(2):
                # broadcast (mu, rstd) to the 16 channels of each group
                pb = pp.tile([128, 2], F32, tag="pp")
                nc.tensor.matmul(pb, (gselT_sb), (mr_t[t]), start=True, stop=True)
                # scale = gnw * rstd ; bias = gnb - mu * scale
                sc = gnp.tile([128, 1], F32, tag=f"sc{t}")
                bi = gnp.tile([128, 1], F32, tag=f"bi{t}")
                nc.vector.tensor_tensor(
                    out=sc, in0=gnw_sb[t], in1=pb[:, 1:2], op=mybir.AluOpType.mult
                )
                nc.vector.tensor_tensor(
                    out=bi, in0=pb[:, 0:1], in1=sc, op=mybir.AluOpType.mult
                )
                nc.vector.tensor_tensor(
                    out=bi, in0=gnb_sb[t], in1=bi, op=mybir.AluOpType.subtract
                )
                s_t.append(sc)
                b_t.append(bi)
            # write xn8 in column-halves so projections can start after the
            # first half instead of waiting for both full-row writes
            for half in range(2):
                for t in range(2):
                    cols = slice(S // 2 * half, S // 2 * (half + 1))
                    nc.vector.tensor_scalar(
                        out=xn8_sb[:, S * t + S // 2 * half : S * t + S // 2 * (half + 1)],
                        in0=x_sb[t][:, cols],
                        scalar1=s_t[t],
                        scalar2=b_t[t],
                        op0=mybir.AluOpType.mult,
                        op1=mybir.AluOpType.add,
                    )

            # ---- projections: fp8 DoubleRow, 256-channel contraction ---
            # K: [256 kch, S];  kch tile t holds heads 2t, 2t+1
            for t in range(2):
                for sb in range(S // 512):
                    ps = pp.tile([128, 512], F32, tag="pp")
                    nc.tensor.matmul(
                        ps,
                        wq8r[:, :, C + 128 * t : C + 128 * (t + 1)],
                        xn8r[:, :, 512 * sb : 512 * (sb + 1)],
                        start=True,
                        stop=True,
                        perf_mode=DR,
                    )
                    nc.scalar.activation(
                        out=k_sb[t][:, 512 * sb : 512 * (sb + 1)], in_=ps, func=AF.Copy
                    )
            # Q: first NQ positions only
            for t in range(2):
                for sb in range(NQ // 512):
                    ps = pp.tile([128, 512], F32, tag="pp")
                    nc.tensor.matmul(
                        ps,
                        wq8r[:, :, 128 * t : 128 * (t + 1)],
                        xn8r[:, :, 512 * sb : 512 * (sb + 1)],
                        start=True,
                        stop=True,
                        perf_mode=DR,
                    )
                    nc.scalar.activation(
                        out=q_sb[t][:, 512 * sb : 512 * (sb + 1)], in_=ps, func=AF.Copy
                    )
            # V^T: [S, 256] in chunks of 128 rows, written into the fp8
            # [128, (j, h, WV)] layout of the chunk-pair tiles.
            for c in range(NCHUNK):
                ps = pp.tile([128, C], F32, tag="pp")
                nc.tensor.matmul(
                    ps,
                    xn8r[:, :, 128 * c : 128 * (c + 1)],
                    wq8r[:, :, 2 * C : 3 * C],
                    start=True,
                    stop=True,
                    perf_mode=DR,
                )
                vdst = vt_sb[c // 2].rearrange("p (j h x) -> p j h x", j=2, h=NH)[
                    :, c % 2, :, 0:HD
                ]
                nc.vector.tensor_copy(out=vdst, in_=ps.rearrange("p (h x) -> p h x", h=NH))

        # ---- attention -------------------------------------------------
        # heads processed in pairs; QK matmuls of the pair target disjoint
        # PE row groups (rows 0-63 / 64-127) and run concurrently.
        # AV is fp8 DoubleRow: 256 keys (one chunk pair) per accumulation
        # pass, e stored as [128, (j, q)] fp8.
        def head_evac(h, po_h):
            nc.vector.tensor_copy(out=oh_sb[h], in_=po_h)

        def pair0_norm_piece(step):
            # pair-0 normalization drip-fed between pair-1 chunks: the
            # [1,1024] DVE reciprocal costs 6.5us (8 cyc/elem on one lane);
            # emitted whole it stalls the strict-FIFO Vector queue and opens
            # a 16us PE bubble at the pair transition. 8 slices of [1,128]
            # fit in the per-chunk DVE slack.
            if 0 <= step < 16:
                i, m = step % 2, step // 2
                nc.vector.reciprocal(
                    out=rs_sb[i][:, 128 * m : 128 * (m + 1)],
                    in_=oh_sb[i][64:65, 128 * m : 128 * (m + 1)],
                )
            elif step == 16:
                for i in range(2):
                    nc.gpsimd.partition_broadcast(
                        rsb_sb[i], rs_sb[i].bitcast(F32), channels=64
                    )
            elif step in (17, 18):
                i = step - 17
                nc.vector.tensor_tensor(
                    out=attn_sb[i],
                    in0=oh_sb[i][0:64, :],
                    in1=rsb_sb[i],
                    op=mybir.AluOpType.mult,
                )

        for pair in range(2):
            po_pair = [
                po.tile([65, NQ], F32, tag="po", name=f"po{pair}_{i}") for i in range(2)
            ]
            for cp in range(NCP):
                e2 = [
                    ep.tile([128, 2 * NQ], FP8, tag=f"e{i}", name=f"e{pair}_{cp}_{i}")
                    for i in range(2)
                ]
                for j in range(2):
                    c = 2 * cp + j
                    # one PSUM bank per (head, query-block): QK of block nb+1
                    # must not wait on the exp drains of block nb
                    ps_q = [
                        [
                            pp.tile([128, 512], F32, tag="pp", name=f"ps{pair}_{c}_{i}_{nb}")
                            for i in range(2)
                        ]
                        for nb in range(NQ // 512)
                    ]
                    for nb in range(NQ // 512):
                        for i in range(2):
                            row = i * 64
                            nc.tensor.matmul(
                                ps_q[nb][i],
                                (k_sb[pair][row : row + 64, 128 * c : 128 * (c + 1)]),
                                (q_sb[pair][row : row + 64, 512 * nb : 512 * (nb + 1)]),
                                start=True,
                                stop=True,
                            )
                        # softmax exp -> fp8: ScalarE (i=0), VectorE (i=1)
                        nc.scalar.activation(
                            out=e2[0][:, NQ * j + 512 * nb : NQ * j + 512 * (nb + 1)],
                            in_=ps_q[nb][0],
                            func=AF.Exp,
                            scale=SCALE,
                        )
                        nc.vector.tensor_scalar(
                            out=e2[1][:, NQ * j + 512 * nb : NQ * j + 512 * (nb + 1)].bitcast(I8),
                            in0=ps_q[nb][1],
                            scalar1=SCH_A,
                            scalar2=SCH_B,
                            op0=mybir.AluOpType.mult,
                            op1=mybir.AluOpType.add,
                        )
                # AV: DoubleRow over the chunk pair (256 keys)
                for i in range(2):
                    h = 2 * pair + i
                    vt3 = vt_sb[cp].rearrange("p (j w) -> p j w", j=2)
                    e3 = e2[i].rearrange("p (j n) -> p j n", j=2)
                    for nb in range(NQ // 512):
                        nc.tensor.matmul(
                            po_pair[i][:, 512 * nb : 512 * (nb + 1)],
                            vt3[:, :, WV * h : WV * h + HD + 1],
                            e3[:, :, 512 * nb : 512 * (nb + 1)],
                            start=(cp == 0),
                            stop=(cp == NCP - 1),
                            perf_mode=DR,
                        )
                if pair == 1:
                    pair0_norm_piece(cp - 1)
            if pair == 0:
                for i in range(2):
                    head_evac(i, po_pair[i])
            else:
                pair0_norm_piece(15)
                pair0_norm_piece(16)
                # pair-1 (tail): 1/d = exp(-ln(d)) on ScalarE straight from
                # PSUM, batched Ln then Exp (2 table-set switches total)
                for i in range(2):
                    nc.scalar.activation(
                        out=ln_sb[i], in_=po_pair[i][64:65, :], func=AF.Ln
                    )
                for i in range(2):
                    head_evac(2 + i, po_pair[i])
                for i in range(2):
                    nc.scalar.activation(
                        out=rs_sb[2 + i], in_=ln_sb[i], func=AF.Exp, scale=-1.0
                    )
                pair0_norm_piece(17)
                pair0_norm_piece(18)

        # ---- normalize pair 1 + out-projection --------------------------
        # out-projection of the pair-0 heads starts while ScalarE finishes
        # the pair-1 reciprocals
        pf_acc = [
            po.tile([128, NQ], F32, tag="po", name=f"pfa{o}") for o in range(2)
        ]

        def emit_outproj(h, start, stop):
            for oct_ in range(2):
                for nb in range(NQ // 512):
                    nc.tensor.matmul(
                        pf_acc[oct_][:, 512 * nb : 512 * (nb + 1)],
                        (wo_sb[h][:, 128 * oct_ : 128 * (oct_ + 1)]),
                        attn_sb[h][:, 512 * nb : 512 * (nb + 1)],
                        start=start,
                        stop=stop,
                    )

        emit_outproj(0, True, False)
        emit_outproj(1, False, False)
        for h in (2, 3):
            # broadcast 1/d via ones-matmul into per-block pp banks
            pbs = []
            for nb in range(NQ // 512):
                pb = pp.tile([64, 512], F32, tag="pp", name=f"pb{h}_{nb}")
                nc.tensor.matmul(
                    pb,
                    ones64_sb,
                    rs_sb[h][:, 512 * nb : 512 * (nb + 1)],
                    start=True,
                    stop=True,
                )
                pbs.append(pb)
            for nb in range(NQ // 512):
                nc.vector.tensor_tensor(
                    out=attn_sb[h][:, 512 * nb : 512 * (nb + 1)],
                    in0=oh_sb[h][0:64, 512 * nb : 512 * (nb + 1)],
                    in1=pbs[nb],
                    op=mybir.AluOpType.mult,
                )
        emit_outproj(2, False, False)
        emit_outproj(3, False, True)
        for oct_ in range(2):
            nc.vector.tensor_tensor(
                out=oacc_sb[oct_],
                in0=oacc_sb[oct_],
                in1=pf_acc[oct_],
                op=mybir.AluOpType.add,
            )

        # ---- store ------------------------------------------------------
        for oct_ in range(2):
            nc.sync.dma_start(out=y_d[128 * oct_ : 128 * (oct_ + 1), :], in_=oacc_sb[oct_])


_PROGRAM = None


def _get_program():
    global _PROGRAM
    if _PROGRAM is None:
        _PROGRAM = _build_program()
    return _PROGRAM


def _prep_inputs(input, gn_weight, gn_bias, qkv_weight, out_weight, out_bias):
    input = np.asarray(input, dtype=np.float32).reshape(B, C, S)
    gn_weight = np.asarray(gn_weight, dtype=np.float32)
    gn_bias = np.asarray(gn_bias, dtype=np.float32)
    qkv_weight = np.asarray(qkv_weight, dtype=np.float32)
    out_weight = np.asarray(out_weight, dtype=np.float32)
    out_bias = np.asarray(out_bias, dtype=np.float32)

    # reference splits qkv head-major: rows 192h..192h+192 = [q|k|v] of head h.
    # device layout wants cols [Q heads 0..3 | K heads 0..3 | V heads 0..3].
    perm = np.concatenate(
        [
            np.arange(192 * h + 64 * part, 192 * h + 64 * (part + 1))
            for part in range(3)
            for h in range(NH)
        ]
    )
    import ml_dtypes

    # fp8 DoubleRow weights: wqkv8[p, j, o] = Wqkv[o, 128j + p]
    wq_perm = np.ascontiguousarray(qkv_weight.T[:, perm])          # [C, 768]
    wqkv8 = np.ascontiguousarray(
        wq_perm.reshape(2, 128, 3 * C).transpose(1, 0, 2).reshape(128, 2 * 3 * C)
    ).astype(ml_dtypes.float8_e4m3fn)
    woutT = np.ascontiguousarray(out_weight.T.reshape(NH, HD, C)).astype(
        ml_dtypes.bfloat16
    )
    gnw = np.ascontiguousarray(gn_weight.reshape(2, 128, 1))
    gnb = np.ascontiguousarray(gn_bias.reshape(2, 128, 1))
    ob = np.ascontiguousarray(out_bias.reshape(2, 128, 1))
    gsel = np.zeros((128, 8), np.float32)
    for p in range(128):
        gsel[p, p // 16] = 1.0
    gselT = np.ascontiguousarray(gsel.T)

    in_maps = []
    for core in range(8):
        b, j = core // 4, core % 4
        xrot = np.roll(input[b], -NQ * j, axis=1)
        in_maps.append(
            {
                "x": np.ascontiguousarray(xrot),
                "wqkv8": wqkv8,
                "woutT": woutT,
                "gnw": gnw,
                "gnb": gnb,
                "ob": ob,
                "gsel": gsel,
                "gselT": gselT,
            }
        )
    return in_maps


def kernel(input, gn_weight, gn_bias, qkv_weight, out_weight, out_bias, _trace=False):
    nc = _get_program()
    in_maps = _prep_inputs(
        input, gn_weight, gn_bias, qkv_weight, out_weight, out_bias
    )
    kw = {}
    if _trace:
        kw = {"trace": True, "tmpdir": "/tmp/attn_trace"}
    res = run_bass_kernel_spmd(nc, in_maps, list(range(8)), **kw)
    out = np.empty((B, C, S), np.float32)
    for core in range(8):
        b, j = core // 4, core % 4
        out[b, :, NQ * j : NQ * (j + 1)] = res.results[core]["y"]
    out = out.reshape(B, C, H, W)
    if _trace:
        return out, res
    return out


# revision 17
# speedup vs baseline: 1.2918x; 1.2918x over previous
"""Spatial attention block (GroupNorm + QKV 1x1 + full spatial attention +
out-proj + residual) on 8 Trainium2 NeuronCores.

Sharding: core = (batch b, spatial quarter j). Each core receives its batch
image rotated along the flattened spatial axis by -1024*j, so the SPMD
program always computes attention outputs for "the first 1024 query
positions" of its input. Attention is invariant to a joint rotation of the
K/V spatial axis, and GroupNorm stats are rotation-invariant, so the host
just concatenates the per-core [256, 1024] outputs.

v2b: fp8(e4m3) everywhere it pays. QKV projections contract 256 channels
per pass via DoubleRow (fp8 xn, fp8 weights); AV contracts 256 keys per
pass via DoubleRow (fp8 e, fp8 [V^T|1]); softmax exp split across ScalarE
(LUT exp -> fp8) and VectorE (Schraudolph int8 bit-trick -> fp8) at
512-column granularity so score PSUM banks recycle fast; reciprocal via
Ln/Exp on ScalarE, batched per head pair to minimize ACT table switches.
"""

import sys

for _p in ("/opt/trn_rl_repo", "/root/.axon_site/_ro/trn_rl_repo"):
    if _p not in sys.path:
        sys.path.insert(0, _p)

import numpy as np

import concourse.bacc as bacc
import concourse.bass as bass
import concourse.tile as tile
from concourse import mybir
from concourse.bass_utils import run_bass_kernel_spmd

F32 = mybir.dt.float32
F32R = mybir.dt.float32r
BF16 = mybir.dt.bfloat16
FP8 = mybir.dt.float8e4
I8 = mybir.dt.int8
U8 = mybir.dt.uint8
AF = mybir.ActivationFunctionType
DR = mybir.MatmulPerfMode.DoubleRow

B, C, H, W = 2, 256, 64, 64
S = H * W              # 4096 spatial positions
NH = 4                 # heads
HD = C // NH           # 64 head dim
NQ = S // 4            # 1024 query positions per core
NCHUNK = S // 128      # 32 key chunks
NCP = NCHUNK // 2      # 16 chunk pairs (DoubleRow contracts 256 keys/pass)
EPS = 1e-5
SCALE = 1.0 / 16.0     # 1/sqrt(C)
WV = 68                # per-head stride in the [V^T | ones] tile (64 V + 1 one + 3 pad)
# Schraudolph int8 fp8e4m3 exp: i8 = round(score * SCH_A + SCH_B)
SCH_A = 8.0 * 1.4426950408889634 / 16.0   # 8*log2(e)*SCALE
SCH_B = 56.0                               # 7 (exp bias) * 8


def _build_program():
    nc = bacc.Bacc(None)

    x_d = nc.declare_dram_parameter("x", [C, S], F32, isOutput=False)
    wqkv8_d = nc.declare_dram_parameter("wqkv8", [128, 2 * 3 * C], FP8, isOutput=False)
    woutT_d = nc.declare_dram_parameter("woutT", [NH, HD, C], BF16, isOutput=False)
    gnw_d = nc.declare_dram_parameter("gnw", [2, 128, 1], F32, isOutput=False)
    gnb_d = nc.declare_dram_parameter("gnb", [2, 128, 1], F32, isOutput=False)
    ob_d = nc.declare_dram_parameter("ob", [2, 128, 1], F32, isOutput=False)
    gsel_d = nc.declare_dram_parameter("gsel", [128, 8], F32R, isOutput=False)
    gselT_d = nc.declare_dram_parameter("gselT", [8, 128], F32R, isOutput=False)
    y_d = nc.declare_dram_parameter("y", [C, NQ], F32, isOutput=True)

    with tile.TileContext(nc) as tc, nc.allow_low_precision("fp8 matmul inputs"):
        _emit(nc, tc, x_d, wqkv8_d, woutT_d, gnw_d, gnb_d, ob_d, gsel_d, gselT_d, y_d)
    nc.finalize()
    return nc


def _emit(nc, tc, x_d, wqkv8_d, woutT_d, gnw_d, gnb_d, ob_d, gsel_d, gselT_d, y_d):
    from contextlib import ExitStack

    ctx = ExitStack()
    with ctx:
        persist = ctx.enter_context(tc.tile_pool(name="persist", bufs=1))
        pp = ctx.enter_context(tc.tile_pool(name="pp", bufs=4, space="PSUM"))
        po = ctx.enter_context(tc.tile_pool(name="po", bufs=2, space="PSUM"))
        ep = ctx.enter_context(tc.tile_pool(name="epool", bufs=2))

        # ---- persistent SBUF tiles -------------------------------------
        x_sb = [persist.tile([128, S], F32, tag=f"x{t}", name=f"x{t}") for t in range(2)]
        k_sb = [persist.tile([128, S], BF16, tag=f"k{t}", name=f"k{t}") for t in range(2)]
        q_sb = [persist.tile([128, NQ], BF16, tag=f"q{t}", name=f"q{t}") for t in range(2)]
        # [V^T | 1] per chunk-pair: cols (j, h*WV + c) j=key subchunk, h=head
        vt_sb = [
            persist.tile([128, 2 * NH * WV], FP8, tag=f"vt{c}", name=f"vt{c}")
            for c in range(NCP)
        ]
        attn_sb = [persist.tile([64, NQ], BF16, tag=f"at{h}", name=f"at{h}") for h in range(NH)]
        wq8_sb = persist.tile([128, 2 * 3 * C], FP8, tag="wq8")
        wo_sb = [persist.tile([HD, C], BF16, tag=f"wo{ct}", name=f"wo{ct}") for ct in range(NH)]
        gnw_sb = [persist.tile([128, 1], F32, tag=f"gw{t}", name=f"gw{t}") for t in range(2)]
        gnb_sb = [persist.tile([128, 1], F32, tag=f"gb{t}", name=f"gb{t}") for t in range(2)]
        ob_sb = [persist.tile([128, 1], F32, tag=f"obias{t}", name=f"obias{t}") for t in range(2)]
        gsel_sb = persist.tile([128, 8], F32R, tag="gsel")
        gselT_sb = persist.tile([8, 128], F32R, tag="gselT")
        oacc_sb = [
            persist.tile([128, NQ], F32, tag=f"oacc{t}", name=f"oacc{t}")
            for t in range(2)
        ]
        oh_sb = [persist.tile([65, NQ], F32, tag=f"oh{h}", name=f"oh{h}") for h in range(NH)]
        rs_sb = [persist.tile([1, NQ], F32R, tag=f"rs{h}", name=f"rs{h}") for h in range(NH)]
        ln_sb = [persist.tile([1, NQ], F32, tag=f"ln{h}", name=f"ln{h}") for h in range(NH)]
        eps_sb = persist.tile([128, 1], F32, tag="eps")
        nc.vector.memset(eps_sb, EPS)
        # memset only supports standard value types: build ones as F32 and
        # bitcast to F32R at the use site.
        ones64_f32 = persist.tile([1, 64], F32, tag="ones64")
        nc.vector.memset(ones64_f32, 1.0)
        ones64_sb = ones64_f32.bitcast(F32R)

        for t in range(2):
            for xc in range(4):
                eng = nc.sync if xc % 2 == 0 else nc.gpsimd
                eng.dma_start(
                    out=x_sb[t][:, 1024 * xc : 1024 * (xc + 1)],
                    in_=x_d[128 * t : 128 * (t + 1), 1024 * xc : 1024 * (xc + 1)],
                )
            nc.sync.dma_start(out=gnw_sb[t], in_=gnw_d[t])
            nc.sync.dma_start(out=gnb_sb[t], in_=gnb_d[t])
            nc.sync.dma_start(out=ob_sb[t], in_=ob_d[t])
        nc.sync.dma_start(out=wq8_sb, in_=wqkv8_d[:, :])
        for ct in range(NH):
            nc.sync.dma_start(out=wo_sb[ct], in_=woutT_d[ct])
        nc.sync.dma_start(out=gsel_sb, in_=gsel_d[:])
        nc.sync.dma_start(out=gselT_sb, in_=gselT_d[:])
        for oct_ in range(2):
            nc.vector.tensor_scalar(
                out=oacc_sb[oct_],
                in0=x_sb[oct_][:, 0:NQ],
                scalar1=ob_sb[oct_],
                scalar2=None,
                op0=mybir.AluOpType.add,
            )

        # ones columns of the [V^T | ones] tiles (0x38 is fp8e4m3 1.0)
        for c in range(NCP):
            ones_cols = vt_sb[c].rearrange("p (j h x) -> p j h x", j=2, h=NH)[
                :, :, :, HD : HD + 1
            ]
            nc.vector.memset(ones_cols.bitcast(U8), 0x38)

        wq8r = wq8_sb.rearrange("p (j o) -> p j o", j=2)

        warm_n = [0]

        def emit_warm(n):
            d = po.tile([128, 512], F32, tag="po", name=f"warm{warm_n[0]}")
            warm_n[0] += 1
            for _ in range(n):
                nc.tensor.matmul(
                    d,
                    wq8r[:, :, 0:128],
                    wq8r[:, :, 0:512],
                    start=True,
                    stop=True,
                    perf_mode=DR,
                )

        emit_warm(12)

        # ---- GroupNorm -------------------------------------------------
        # per-channel stats via bn_stats (free-dim), then combine the 16
        # channels of each group across partitions with small PE matmuls.
        # Ln/Exp batched across both channel-halves (one table-set trip).
        with tc.tile_pool(name="gnpool", bufs=1) as gnp, tc.tile_pool(
            name="xn", bufs=1
        ) as xnp:
            xn8_sb = xnp.tile([128, 2 * S], FP8, tag="xn8")
            xn8r = xn8_sb.rearrange("p (j s) -> p j s", j=2)
            vg_t, mr_t = [], []
            for t in range(2):
                nsub = S // 512
                st6 = gnp.tile([128, nsub, 6], F32, tag=f"st6_{t}")
                for i in range(nsub):
                    nc.vector.bn_stats(
                        out=st6[:, i, :], in_=x_sb[t][:, 512 * i : 512 * (i + 1)]
                    )
                mv = gnp.tile([128, 2], F32, tag=f"mv{t}")
                nc.vector.bn_aggr(out=mv, in_=st6)
                # stats2 = [mean, var + mean^2]  (per channel)
                stats2 = gnp.tile([128, 2], F32R, tag=f"s2_{t}")
                nc.vector.tensor_copy(out=stats2[:, 0:1], in_=mv[:, 0:1])
                nc.vector.tensor_tensor(
                    out=stats2[:, 1:2],
                    in0=mv[:, 0:1],
                    in1=mv[:, 0:1],
                    op=mybir.AluOpType.mult,
                )
                nc.vector.tensor_tensor(
                    out=stats2[:, 1:2],
                    in0=stats2[:, 1:2],
                    in1=mv[:, 1:2],
                    op=mybir.AluOpType.add,
                )
                # group sums: [8, 2] = gsel.T @ stats2, then /16
                pg = pp.tile([8, 2], F32, tag="pp")
                nc.tensor.matmul(pg, (gsel_sb), (stats2), start=True, stop=True)
                g2 = gnp.tile([8, 2], F32, tag=f"g2_{t}")
                nc.scalar.activation(out=g2, in_=pg, func=AF.Copy, scale=1.0 / 16.0)
                # var_g = m2_g - mu_g^2
                mr = gnp.tile([8, 2], F32R, tag=f"mr{t}")
                nc.vector.tensor_copy(out=mr[:, 0:1], in_=g2[:, 0:1])
                vg = gnp.tile([8, 1], F32, tag=f"vg{t}")
                nc.vector.tensor_tensor(
                    out=vg, in0=g2[:, 0:1], in1=g2[:, 0:1], op=mybir.AluOpType.mult
                )
                nc.vector.tensor_tensor(
                    out=vg, in0=g2[:, 1:2], in1=vg, op=mybir.AluOpType.subtract
                )
                vg_t.append(vg)
                mr_t.append(mr)
            emit_warm(8)
            # rstd = exp(-0.5*ln(var+eps)); batch Ln then Exp
            for t in range(2):
                nc.scalar.activation(out=vg_t[t], in_=vg_t[t], func=AF.Ln, bias=eps_sb[0:8])
            for t in range(2):
                nc.scalar.activation(out=mr_t[t][:, 1:2], in_=vg_t[t], func=AF.Exp, scale=-0.5)
            s_t, b_t = [], []
            for t in range(2):
                # broadcast (mu, rstd) to the 16 channels of each group
                pb = pp.tile([128, 2], F32, tag="pp")
                nc.tensor.matmul(pb, (gselT_sb), (mr_t[t]), start=True, stop=True)
                # scale = gnw * rstd ; bias = gnb - mu * scale
                sc = gnp.tile([128, 1], F32, tag=f"sc{t}")
                bi = gnp.tile([128, 1], F32, tag=f"bi{t}")
                nc.vector.tensor_tensor(
                    out=sc, in0=gnw_sb[t], in1=pb[:, 1:2], op=mybir.AluOpType.mult
                )
                nc.vector.tensor_tensor(
                    out=bi, in0=pb[:, 0:1], in1=sc, op=mybir.AluOpType.mult
                )
                nc.vector.tensor_tensor(
                    out=bi, in0=gnb_sb[t], in1=bi, op=mybir.AluOpType.subtract
                )
                s_t.append(sc)
                b_t.append(bi)
            # write xn8 in column-halves so projections can start after the
            # first half instead of waiting for both full-row writes
            for half in range(2):
                for t in range(2):
                    cols = slice(S // 2 * half, S // 2 * (half + 1))
                    nc.vector.tensor_scalar(
                        out=xn8_sb[:, S * t + S // 2 * half : S * t + S // 2 * (half + 1)],
                        in0=x_sb[t][:, cols],
                        scalar1=s_t[t],
                        scalar2=b_t[t],
                        op0=mybir.AluOpType.mult,
                        op1=mybir.AluOpType.add,
                    )

            # ---- projections: fp8 DoubleRow, 256-channel contraction ---
            # K: [256 kch, S];  kch tile t holds heads 2t, 2t+1
            for t in range(2):
                for sb in range(S // 512):
                    ps = pp.tile([128, 512], F32, tag="pp")
                    nc.tensor.matmul(
                        ps,
                        wq8r[:, :, C + 128 * t : C + 128 * (t + 1)],
                        xn8r[:, :, 512 * sb : 512 * (sb + 1)],
                        start=True,
                        stop=True,
                        perf_mode=DR,
                    )
                    nc.scalar.activation(
                        out=k_sb[t][:, 512 * sb : 512 * (sb + 1)], in_=ps, func=AF.Copy
                    )
            # Q: first NQ positions only
            for t in range(2):
                for sb in range(NQ // 512):
                    ps = pp.tile([128, 512], F32, tag="pp")
                    nc.tensor.matmul(
                        ps,
                        wq8r[:, :, 128 * t : 128 * (t + 1)],
                        xn8r[:, :, 512 * sb : 512 * (sb + 1)],
                        start=True,
                        stop=True,
                        perf_mode=DR,
                    )
                    nc.scalar.activation(
                        out=q_sb[t][:, 512 * sb : 512 * (sb + 1)], in_=ps, func=AF.Copy
                    )
            # V^T: [S, 256] in chunks of 128 rows, written into the fp8
            # [128, (j, h, WV)] layout of the chunk-pair tiles.
            for c in range(NCHUNK):
                ps = pp.tile([128, C], F32, tag="pp")
                nc.tensor.matmul(
                    ps,
                    xn8r[:, :, 128 * c : 128 * (c + 1)],
                    wq8r[:, :, 2 * C : 3 * C],
                    start=True,
                    stop=True,
                    perf_mode=DR,
                )
                vdst = vt_sb[c // 2].rearrange("p (j h x) -> p j h x", j=2, h=NH)[
                    :, c % 2, :, 0:HD
                ]
                nc.vector.tensor_copy(out=vdst, in_=ps.rearrange("p (h x) -> p h x", h=NH))

        # ---- attention -------------------------------------------------
        # heads processed in pairs; QK matmuls of the pair target disjoint
        # PE row groups (rows 0-63 / 64-127) and run concurrently.
        # AV is fp8 DoubleRow: 256 keys (one chunk pair) per accumulation
        # pass, e stored as [128, (j, q)] fp8.
        def head_evac(h, po_h):
            nc.vector.tensor_copy(out=oh_sb[h], in_=po_h)

        for pair in range(2):
            po_pair = [
                po.tile([65, NQ], F32, tag="po", name=f"po{pair}_{i}") for i in range(2)
            ]
            for cp in range(NCP):
                e2 = [
                    ep.tile([128, 2 * NQ], FP8, tag=f"e{i}", name=f"e{pair}_{cp}_{i}")
                    for i in range(2)
                ]
                for j in range(2):
                    c = 2 * cp + j
                    # one PSUM bank per (head, query-block): QK of block nb+1
                    # must not wait on the exp drains of block nb
                    ps_q = [
                        [
                            pp.tile([128, 512], F32, tag="pp", name=f"ps{pair}_{c}_{i}_{nb}")
                            for i in range(2)
                        ]
                        for nb in range(NQ // 512)
                    ]
                    for nb in range(NQ // 512):
                        for i in range(2):
                            row = i * 64
                            nc.tensor.matmul(
                                ps_q[nb][i],
                                (k_sb[pair][row : row + 64, 128 * c : 128 * (c + 1)]),
                                (q_sb[pair][row : row + 64, 512 * nb : 512 * (nb + 1)]),
                                start=True,
                                stop=True,
                            )
                        # softmax exp -> fp8: ScalarE (i=0), VectorE (i=1)
                        nc.scalar.activation(
                            out=e2[0][:, NQ * j + 512 * nb : NQ * j + 512 * (nb + 1)],
                            in_=ps_q[nb][0],
                            func=AF.Exp,
                            scale=SCALE,
                        )
                        nc.vector.tensor_scalar(
                            out=e2[1][:, NQ * j + 512 * nb : NQ * j + 512 * (nb + 1)].bitcast(I8),
                            in0=ps_q[nb][1],
                            scalar1=SCH_A,
                            scalar2=SCH_B,
                            op0=mybir.AluOpType.mult,
                            op1=mybir.AluOpType.add,
                        )
                # AV: DoubleRow over the chunk pair (256 keys)
                for i in range(2):
                    h = 2 * pair + i
                    vt3 = vt_sb[cp].rearrange("p (j w) -> p j w", j=2)
                    e3 = e2[i].rearrange("p (j n) -> p j n", j=2)
                    for nb in range(NQ // 512):
                        nc.tensor.matmul(
                            po_pair[i][:, 512 * nb : 512 * (nb + 1)],
                            vt3[:, :, WV * h : WV * h + HD + 1],
                            e3[:, :, 512 * nb : 512 * (nb + 1)],
                            start=(cp == 0),
                            stop=(cp == NCP - 1),
                            perf_mode=DR,
                        )
            for i in range(2):
                head_evac(2 * pair + i, po_pair[i])

        # ---- reciprocals + normalize + out-projection --------------------
        # 1/d = exp(-ln(d)) on ScalarE for all 4 heads, Ln batch then Exp
        # batch: exactly 2 table-set switches, nothing mid-loop.
        for h in range(NH):
            nc.scalar.activation(out=ln_sb[h], in_=oh_sb[h][64:65, :], func=AF.Ln)
        for h in range(NH):
            nc.scalar.activation(out=rs_sb[h], in_=ln_sb[h], func=AF.Exp, scale=-1.0)

        pf_acc = [
            po.tile([128, NQ], F32, tag="po", name=f"pfa{o}") for o in range(2)
        ]
        for h in range(NH):
            # broadcast 1/d via ones-matmul into per-block pp banks
            pbs = []
            for nb in range(NQ // 512):
                pb = pp.tile([64, 512], F32, tag="pp", name=f"pb{h}_{nb}")
                nc.tensor.matmul(
                    pb,
                    ones64_sb,
                    rs_sb[h][:, 512 * nb : 512 * (nb + 1)],
                    start=True,
                    stop=True,
                )
                pbs.append(pb)
            for nb in range(NQ // 512):
                nc.vector.tensor_tensor(
                    out=attn_sb[h][:, 512 * nb : 512 * (nb + 1)],
                    in0=oh_sb[h][0:64, 512 * nb : 512 * (nb + 1)],
                    in1=pbs[nb],
                    op=mybir.AluOpType.mult,
                )
            for oct_ in range(2):
                for nb in range(NQ // 512):
                    nc.tensor.matmul(
                        pf_acc[oct_][:, 512 * nb : 512 * (nb + 1)],
                        (wo_sb[h][:, 128 * oct_ : 128 * (oct_ + 1)]),
                        attn_sb[h][:, 512 * nb : 512 * (nb + 1)],
                        start=(h == 0),
                        stop=(h == NH - 1),
                    )
        for oct_ in range(2):
            nc.vector.tensor_tensor(
                out=oacc_sb[oct_],
                in0=oacc_sb[oct_],
                in1=pf_acc[oct_],
                op=mybir.AluOpType.add,
            )

        # ---- store ------------------------------------------------------
        for oct_ in range(2):
            nc.sync.dma_start(out=y_d[128 * oct_ : 128 * (oct_ + 1), :], in_=oacc_sb[oct_])


_PROGRAM = None


def _get_program():
    global _PROGRAM
    if _PROGRAM is None:
        _PROGRAM = _build_program()
    return _PROGRAM


def _prep_inputs(input, gn_weight, gn_bias, qkv_weight, out_weight, out_bias):
    input = np.asarray(input, dtype=np.float32).reshape(B, C, S)
    gn_weight = np.asarray(gn_weight, dtype=np.float32)
    gn_bias = np.asarray(gn_bias, dtype=np.float32)
    qkv_weight = np.asarray(qkv_weight, dtype=np.float32)
    out_weight = np.asarray(out_weight, dtype=np.float32)
    out_bias = np.asarray(out_bias, dtype=np.float32)

    # reference splits qkv head-major: rows 192h..192h+192 = [q|k|v] of head h.
    # device layout wants cols [Q heads 0..3 | K heads 0..3 | V heads 0..3].
    perm = np.concatenate(
        [
            np.arange(192 * h + 64 * part, 192 * h + 64 * (part + 1))
            for part in range(3)
            for h in range(NH)
        ]
    )
    import ml_dtypes

    # fp8 DoubleRow weights: wqkv8[p, j, o] = Wqkv[o, 128j + p]
    wq_perm = np.ascontiguousarray(qkv_weight.T[:, perm])          # [C, 768]
    wqkv8 = np.ascontiguousarray(
        wq_perm.reshape(2, 128, 3 * C).transpose(1, 0, 2).reshape(128, 2 * 3 * C)
    ).astype(ml_dtypes.float8_e4m3fn)
    woutT = np.ascontiguousarray(out_weight.T.reshape(NH, HD, C)).astype(
        ml_dtypes.bfloat16
    )
    gnw = np.ascontiguousarray(gn_weight.reshape(2, 128, 1))
    gnb = np.ascontiguousarray(gn_bias.reshape(2, 128, 1))
    ob = np.ascontiguousarray(out_bias.reshape(2, 128, 1))
    gsel = np.zeros((128, 8), np.float32)
    for p in range(128):
        gsel[p, p // 16] = 1.0
    gselT = np.ascontiguousarray(gsel.T)

    in_maps = []
    for core in range(8):
        b, j = core // 4, core % 4
        xrot = np.roll(input[b], -NQ * j, axis=1)
        in_maps.append(
            {
                "x": np.ascontiguousarray(xrot),
                "wqkv8": wqkv8,
                "woutT": woutT,
                "gnw": gnw,
                "gnb": gnb,
                "ob": ob,
                "gsel": gsel,
                "gselT": gselT,
            }
        )
    return in_maps


def kernel(input, gn_weight, gn_bias, qkv_weight, out_weight, out_bias, _trace=False):
    nc = _get_program()
    in_maps = _prep_inputs(
        input, gn_weight, gn_bias, qkv_weight, out_weight, out_bias
    )
    kw = {}
    if _trace:
        kw = {"trace": True, "tmpdir": "/tmp/attn_trace"}
    res = run_bass_kernel_spmd(nc, in_maps, list(range(8)), **kw)
    out = np.empty((B, C, S), np.float32)
    for core in range(8):
        b, j = core // 4, core % 4
        out[b, :, NQ * j : NQ * (j + 1)] = res.results[core]["y"]
    out = out.reshape(B, C, H, W)
    if _trace:
        return out, res
    return out
